# revision 37
# baseline (speedup 1.0000x reference)
"""TRN2 Bass/Tile kernel for AttentionBlock: GroupNorm(32) + 1x1-conv QKV +
single-head softmax attention over N=H*W tokens + output proj + residual.

Sharding: 8 cores = 4 samples x 2 query-halves (data parallel over batch,
query-parallel within sample). Each core receives the full (row-permuted)
sample so it can compute K/V for all 4096 tokens, but computes Q / attention /
output only for its 2048 query rows. No collectives needed.

v3: all four big GEMMs (Q-proj, V-proj, scores, PV) are fp8 (e4m3) DoubleRow
matmuls (two 128-deep k-planes per instruction, 0.5 cycles per output
column). The host supplies raw x in fp8 twice — channel-major x^T for GEMM
operands and token-major rows for statistics — plus x16-scaled fp8 folded
weights (wq@wk^T, wv@wo).

GroupNorm statistics run on the tensor engine while input DMAs stream:
sum(x) via an all-ones moving operand, sum(x^2) as the diagonal of per-chunk
Gram matrices X^T X (extracted with an identity mask + free-axis reduce).
rstd = exp(-0.5*ln(var+eps)) keeps the whole kernel on one activation table
(ln/exp/identity). The affine+fp8 quantize pass is token-major on DVE+Pool
so consumers start after the first 512 tokens.

Attention is a software pipeline over 512-query tiles: scores land in
two-bank PSUM pairs (one 1024-wide exp per pair keeps ACT saturated); P@V
for tile qt runs as a no-wait burst early in tile qt+1's window (qt3: second
half of its own window) into four persistent PSUM chains; the softmax
denominator is four tiny DoubleRow chains against a 16.0-constant moving
operand (the 16 folds the fp8 weight scale) read back per query partition
with a single reciprocal — no transposes. Normalization (1/(16 l)) and the
bf16 residual add fuse into one scalar_tensor_tensor per sub-tile.
"""

import math

import numpy as np
import ml_dtypes

B, H, W, C = 4, 64, 64, 512
N = H * W            # 4096 tokens per sample
NQ = N // 2          # 2048 query rows per core
GROUPS = 32
GSIZE = C // GROUPS  # 16 channels per group
EPS = 1e-5
NCORES = 8
KBLK = 512           # query-tile / psum free size
CCH = C // 128       # 4 channel chunks
NKC = N // 128       # 32 key chunks
NKP = NKC // 2       # 16 key chunk pairs
NQT = NQ // KBLK     # 4 query tiles
WSC = 16.0           # fp8 weight scale
EXP_SCALE = 1.0 / (WSC * math.sqrt(C))
EXP_BIAS = -2.0      # cancels in softmax; keeps exp() inside fp8e4 range
ONES_W = 128         # moving-operand width for the sum chains

_BUILD_CACHE = {}


def _build_nc():
    import concourse.bass as bass
    import concourse.tile as tile
    from concourse import bacc, mybir

    f32 = mybir.dt.float32
    bf16 = mybir.dt.bfloat16
    f8 = mybir.dt.float8e4
    Alu = mybir.AluOpType
    Act = mybir.ActivationFunctionType
    DR = mybir.MatmulPerfMode.DoubleRow

    nc = bacc.Bacc("TRN2", target_bir_lowering=False, debug=False,
                   num_devices=NCORES)

    xt_d = nc.dram_tensor("xt", [8, 128, CCH, 512], f8, kind="ExternalInput")
    xrow_d = nc.dram_tensor("xrow", [128, NKC, C], f8, kind="ExternalInput")
    xr_d = nc.dram_tensor("xr", [NQ, C], bf16, kind="ExternalInput")
    wq_d = nc.dram_tensor("wq", [128, CCH, C], f8, kind="ExternalInput")
    wv_d = nc.dram_tensor("wv", [128, CCH, C], f8, kind="ExternalInput")
    gamma_d = nc.dram_tensor("gamma", [C], f32, kind="ExternalInput")
    beta_d = nc.dram_tensor("beta", [C], f32, kind="ExternalInput")
    # gmat is pre-scaled by 1/(GSIZE*N) so the group matmul yields means
    gmat_d = nc.dram_tensor("gmat", [128, 8], f32, kind="ExternalInput")
    gtmat_d = nc.dram_tensor("gtmat", [8, 128], f32, kind="ExternalInput")
    imat_d = nc.dram_tensor("imat", [128, 128], bf16, kind="ExternalInput")
    out_d = nc.dram_tensor("out", [NQ, C], bf16, kind="ExternalOutput")

    with tile.TileContext(nc) as tc:
        with (
            tc.tile_pool(name="big", bufs=1) as big,
            tc.tile_pool(name="wpool", bufs=1) as wpool,
            tc.tile_pool(name="stats", bufs=1) as stats,
            tc.tile_pool(name="tmp", bufs=3) as tmp,
            tc.tile_pool(name="ptile", bufs=2) as ptile,
            tc.tile_pool(name="small", bufs=2) as small,
            tc.tile_pool(name="pp", bufs=2, space="PSUM") as pp,
            tc.tile_pool(name="pvA", bufs=1, space="PSUM") as pvA,
            tc.tile_pool(name="pvB", bufs=1, space="PSUM") as pvB,
        ):
            # ---- resident tensors (per-block tiles so consumers start as
            # soon as their block's DMA/affine lands) ----
            NTB = 8
            TB = N // NTB
            xt8_t = [big.tile([128, CCH, TB], f8, tag=f"xt8_{i}",
                              name=f"xt8_{i}") for i in range(NTB)]
            xrow8_h = [big.tile([128, NKC // 2, C], f8, tag=f"xrow8_{i}",
                                name=f"xrow8_{i}") for i in range(2)]
            xn8_t = [big.tile([128, CCH, TB], f8, tag=f"xn8_{i}",
                              name=f"xn8_{i}") for i in range(NTB)]

            def xn8s(ci0, ci1, n0, n1):
                t = n0 // TB
                assert n1 <= (t + 1) * TB
                return xn8_t[t][:, ci0:ci1, n0 - t * TB:n1 - t * TB]
            qt8 = big.tile([128, CCH, NQ], f8, tag="qt8")
            v8 = big.tile([128, NKC, C], f8, tag="v8")
            xr_sb = big.tile([128, NQ // 128, C], bf16, tag="xr")

            gamma_sb = wpool.tile([128, CCH], f32, tag="gamma")
            beta_sb = wpool.tile([128, CCH], f32, tag="beta")
            g_sb = wpool.tile([128, 8], f32, tag="gmat")
            gt_sb = wpool.tile([8, 128], f32, tag="gtmat")
            i_sb = wpool.tile([128, 128], bf16, tag="imat")
            nc.sync.dma_start(out=gamma_sb[:, :],
                              in_=gamma_d.ap().rearrange("(a b) -> b a", b=128))
            nc.sync.dma_start(out=beta_sb[:, :],
                              in_=beta_d.ap().rearrange("(a b) -> b a", b=128))
            nc.sync.dma_start(out=g_sb[:, :], in_=gmat_d[:, :])
            nc.sync.dma_start(out=gt_sb[:, :], in_=gtmat_d[:, :])
            nc.sync.dma_start(out=i_sb[:, :], in_=imat_d[:, :])

            # constants + ln/exp activation-table preload while DMAs run
            eps8 = wpool.tile([8, 1], f32, tag="eps")
            nc.vector.memset(eps8[:, :], EPS)
            bneg2 = wpool.tile([128, 1], f32, tag="bneg2")
            nc.vector.memset(bneg2[:, :], EXP_BIAS)
            ones1m = wpool.tile([128, 2, ONES_W], f8, tag="ones1m")
            nc.vector.memset(ones1m[:, :, :], 1.0)
            ones16m = wpool.tile([128, 2, ONES_W], f8, tag="ones16m")
            nc.vector.memset(ones16m[:, :, :], WSC)
            tjunk = wpool.tile([8, 1], f32, tag="tjunk")
            nc.scalar.activation(out=tjunk[:, :], in_=eps8[:, :], func=Act.Ln,
                                 bias=eps8[:, :], scale=1.0)

            # ---- input DMAs in shared-DMA-device priority order ----
            # xrow first (stats), then x^T token-blocks (affine), weights, xr
            qs_ = (nc.sync, nc.scalar)
            for hh in range(2):
                qs_[hh].dma_start(
                    out=xrow8_h[hh][:, :, :],
                    in_=xrow_d[:, hh * (NKC // 2):(hh + 1) * (NKC // 2), :])
            for tb in range(NTB):
                qs_[tb % 2].dma_start(out=xt8_t[tb][:, :, :],
                                      in_=xt_d[tb, :, :, :])
            w8 = {}
            for qi, (name, wd) in enumerate((("wq", wq_d), ("wv", wv_d))):
                w8[name] = wpool.tile([128, CCH, C], f8, tag=name,
                                      name=f"w_{name}")
                qs_[qi].dma_start(out=w8[name][:, :, :], in_=wd[:, :, :])
            for i in range(4):
                qs_[i % 2].dma_start(
                    out=xr_sb[:, i * 4:(i + 1) * 4, :],
                    in_=xr_d.ap().rearrange("(a b) d -> b a d", b=128)[
                        :, i * 4:(i + 1) * 4, :])

            # ---- GroupNorm statistics on the PE while DMAs stream ----
            # sum(x): ones-moving DoubleRow chains; sum(x^2): Gram diagonals.
            # Each chunk chain owns one psum bank (a pool-tile half).
            mv2 = stats.tile([128, CCH, 2], f32, tag="mv2")  # (sum, sumsq)
            sxt = [pp.tile([128, 2, KBLK], f32, tag="pp", name=f"sx{i}")
                   for i in range(2)]
            gmt = [pvA.tile([128, 2, KBLK], f32, tag="pvA", name="gram01"),
                   pvB.tile([128, 2, KBLK], f32, tag="pvB", name="gram23")]
            for cc in range(CCH):
                psx = sxt[cc // 2][:, cc % 2, 0:ONES_W]
                psg = gmt[cc // 2][:, cc % 2, 0:128]
                for nbp in range(NKP):
                    hh, off = divmod(2 * nbp, NKC // 2)
                    lhs = xrow8_h[hh][:, off:off + 2,
                                      cc * 128:(cc + 1) * 128]
                    nc.tensor.matmul(psx, lhs, ones1m[:, :, :],
                                     start=(nbp == 0), stop=(nbp == NKP - 1),
                                     perf_mode=DR)
                    nc.tensor.matmul(psg, lhs, lhs,
                                     start=(nbp == 0), stop=(nbp == NKP - 1),
                                     perf_mode=DR)
            djunk = stats.tile([128, 128], f32, tag="djunk")
            for cc in range(CCH):
                nc.vector.tensor_copy(mv2[:, cc, 0:1],
                                      sxt[cc // 2][:, cc % 2, 0:1])
                nc.vector.tensor_tensor(out=djunk[:, :],
                                        in0=gmt[cc // 2][:, cc % 2, 0:128],
                                        in1=i_sb[:, :], op=Alu.mult)
                nc.vector.tensor_reduce(out=mv2[:, cc, 1:2], in_=djunk[:, :],
                                        axis=mybir.AxisListType.X, op=Alu.add)

            # group combine: per-cc [8,2] matmuls, one bank per chain; the
            # host-scaled gmat turns sums directly into (mean, E[x^2])
            cmb = [pp.tile([128, 2, KBLK], f32, tag="pp", name=f"cmb{i}")
                   for i in range(2)]
            sg = stats.tile([8, CCH, 2], f32, tag="sg")
            for cc in range(CCH):
                pcc = cmb[cc // 2][0:8, cc % 2, 0:2]
                nc.tensor.matmul(pcc, g_sb[:, :], mv2[:, cc, :],
                                 start=True, stop=True)
                nc.vector.tensor_copy(sg[:, cc, :], pcc)
            gv = stats.tile([8, CCH], f32, tag="gv")
            nc.vector.tensor_mul(gv[:, :], sg[:, :, 0], sg[:, :, 0])
            nc.vector.tensor_sub(gv[:, :], sg[:, :, 1], gv[:, :])
            # rstd = exp(-0.5*ln(var+eps)) — stays on the ln/exp ACT table
            nc.scalar.activation(out=gv[:, :], in_=gv[:, :], func=Act.Ln,
                                 bias=eps8[:, :], scale=1.0)
            nc.scalar.activation(out=sg[:, :, 1], in_=gv[:, :], func=Act.Exp,
                                 scale=-0.5)
            # broadcast mean/rstd back to channel partitions (two chains)
            mbp = pvA.tile([128, 2, KBLK], f32, tag="pvA", name="mb")
            nc.tensor.matmul(mbp[:, 0, 0:CCH], gt_sb[:, :], sg[:, :, 0],
                             start=True, stop=True)
            nc.tensor.matmul(mbp[:, 1, 0:CCH], gt_sb[:, :], sg[:, :, 1],
                             start=True, stop=True)
            a_sb = stats.tile([128, CCH], f32, tag="A")
            b_sb = stats.tile([128, CCH], f32, tag="Bb")
            nc.vector.tensor_mul(a_sb[:, :], mbp[:, 1, 0:CCH], gamma_sb[:, :])
            nc.vector.tensor_mul(b_sb[:, :], mbp[:, 0, 0:CCH], a_sb[:, :])
            nc.vector.tensor_sub(b_sb[:, :], beta_sb[:, :], b_sb[:, :])

            # keep the PE p-state warm between the stats chains and the
            # first score matmuls (dummy DoubleRow chain, no consumers)
            warm = pvA.tile([128, 2, KBLK], f32, tag="pvA", name="warm")
            for i in range(20):
                nc.tensor.matmul(warm[:, 0, :],
                                 xrow8_h[0][:, 0:2, 0:128],
                                 xrow8_h[0][:, 0:2, :],
                                 start=(i == 0), stop=(i == 19),
                                 perf_mode=DR)

            # ---- affine + fp8 quantize, token-major, Pool-heavy so DVE
            # stays free for the projection-psum copies ----
            for tb in range(NTB):
                for cc in range(CCH):
                    eng = nc.vector if cc == 0 else nc.gpsimd
                    eng.tensor_scalar(
                        out=xn8_t[tb][:, cc, :], in0=xt8_t[tb][:, cc, :],
                        scalar1=a_sb[:, cc:cc + 1],
                        scalar2=b_sb[:, cc:cc + 1],
                        op0=Alu.mult, op1=Alu.add)

            # ---- helper emitters (DoubleRow fp8 everywhere) ----
            # Projection psums go through the pvA/pvB rings (idle until the
            # first P@V in window 1) so the scores/exp stream owns the pp
            # ring exclusively; each pair-tile is drained by ONE wide copy.
            def qproj(qtile, half, pool, tag, on_act=False):
                """Project 512 queries for dc chunks (2*half, 2*half+1)."""
                q0 = qtile * KBLK
                pt_ = pool.tile([128, 2, KBLK], f32, tag=tag,
                                name=f"qp{qtile}_{half}")
                for h2 in range(2):
                    dc = 2 * half + h2
                    for ci in range(0, CCH, 2):
                        nc.tensor.matmul(
                            pt_[:, h2, :],
                            w8["wq"][:, ci:ci + 2,
                                     dc * 128:(dc + 1) * 128],
                            xn8s(ci, ci + 2, q0, q0 + KBLK),
                            start=(ci == 0), stop=(ci == CCH - 2),
                            perf_mode=DR)
                dst = qt8[:, 2 * half:2 * half + 2, q0:q0 + KBLK]
                if on_act:
                    nc.scalar.activation(out=dst, in_=pt_[:, :, :],
                                         func=Act.Identity)
                else:
                    nc.vector.tensor_copy(dst, pt_[:, :, :])

            def vproj_pair(i, pool, tag):
                """V-projection for token blocks 2i, 2i+1 (halves of a tile)."""
                pt_ = pool.tile([128, 2, KBLK], f32, tag=tag,
                                name=f"vp{i}")
                for h2 in range(2):
                    nb = 2 * i + h2
                    for ci in range(0, CCH, 2):
                        nc.tensor.matmul(
                            pt_[:, h2, :],
                            xn8s(ci, ci + 2, nb * 128, (nb + 1) * 128),
                            w8["wv"][:, ci:ci + 2, :],
                            start=(ci == 0), stop=(ci == CCH - 2),
                            perf_mode=DR)
                nc.vector.tensor_copy(v8[:, 2 * i:2 * i + 2, :],
                                      pt_[:, :, :])

            def pv_steps(pt8_t, psA, psB, kps):
                """P@V chain steps (4 sub chains, kp-major)."""
                for kp in kps:
                    for sub in range(CCH):
                        dst = (psA, psB)[sub // 2][:, sub % 2, :]
                        nc.tensor.matmul(
                            dst,
                            pt8_t[:, 2 * kp:2 * kp + 2,
                                  sub * 128:(sub + 1) * 128],
                            v8[:, 2 * kp:2 * kp + 2, :],
                            start=(kp == 0), stop=(kp == NKP - 1),
                            perf_mode=DR)

            def epilogue(qt, rq, psA, psB):
                q0 = qt * KBLK
                for sub in range(CCH):
                    src = (psA, psB)[sub // 2][:, sub % 2, :]
                    res = tmp.tile([128, C], bf16, tag="res",
                                   name=f"res{qt}_{sub}")
                    nc.vector.scalar_tensor_tensor(
                        out=res[:, :], in0=src,
                        scalar=rq[:, sub:sub + 1],
                        in1=xr_sb[:, qt * 4 + sub, :],
                        op0=Alu.mult, op1=Alu.add)
                    qs = slice(q0 + sub * 128, q0 + (sub + 1) * 128)
                    nc.sync.dma_start(out=out_d[qs, :], in_=res[:, :])

            # ---- attention: software pipeline over query tiles ----
            qproj(0, 0, pp, "pp", on_act=True)
            qproj(0, 1, pp, "pp", on_act=True)
            # filler queue: projection tiles drained through the pvA/pvB
            # rings during windows 0-1 (one per slot, alternating rings)
            fillers = []
            for qtile in (1, 2, 3):
                for half in range(2):
                    fillers.append(("q", qtile, half))
            for i in range(NKC // 2):
                fillers.append(("v", i))
            fi = 0
            pt8_t = {}
            rq_t = {}
            pv_t = {}
            for qt in range(NQT):
                q0 = qt * KBLK
                pt8_t[qt] = ptile.tile([128, NKC, KBLK], f8, tag="pt",
                                       name=f"pt{qt}")
                for kp in range(NKP):
                    # PV step for the previous tile first (always ready)
                    if qt >= 1:
                        prev = qt - 1
                        if kp == 0:
                            pv_t[prev] = (
                                pvA.tile([128, 2, KBLK], f32, tag="pvA",
                                         name=f"pva{prev}"),
                                pvB.tile([128, 2, KBLK], f32, tag="pvB",
                                         name=f"pvb{prev}"))
                        pv_steps(pt8_t[prev], *pv_t[prev], [kp])

                    ppt = pp.tile([128, 2, KBLK], f32, tag="pp",
                                  name=f"s{qt}_{kp}")
                    for half in range(2):
                        kc = 2 * kp + half
                        for ci in range(0, CCH, 2):
                            nc.tensor.matmul(
                                ppt[:, half, :],
                                xn8s(ci, ci + 2, kc * 128, (kc + 1) * 128),
                                qt8[:, ci:ci + 2, q0:q0 + KBLK],
                                start=(ci == 0), stop=(ci == CCH - 2),
                                perf_mode=DR)
                    nc.scalar.activation(out=pt8_t[qt][:, 2 * kp:2 * kp + 2, :],
                                         in_=ppt[:, :, :], func=Act.Exp,
                                         scale=EXP_SCALE, bias=bneg2[:, :])

                    # one filler tile per slot while windows 0-1 have psum
                    # slack in the PV rings
                    if qt == 0 and fi < len(fillers):
                        for _ in range(2 if kp < 6 else 1):
                            if fi < len(fillers):
                                f = fillers[fi]
                                pool, tag = ((pvA, "pvA") if fi % 2 == 0
                                             else (pvB, "pvB"))
                                if f[0] == "q":
                                    qproj(f[1], f[2], pool, tag)
                                else:
                                    vproj_pair(f[1], pool, tag)
                                fi += 1
                    if qt >= 1 and kp == NKP - 1:
                        epilogue(qt - 1, rq_t[qt - 1], *pv_t[qt - 1])

                # denominator: four tiny chains against the 16.0 operand,
                # emitted at tile end (all exps of qt done shortly after)
                dn = [pp.tile([128, 2, KBLK], f32, tag="pp",
                              name=f"dn{qt}_{i}") for i in range(2)]
                for sub in range(CCH):
                    dst = dn[sub // 2][:, sub % 2, 0:ONES_W]
                    for kp in range(NKP):
                        nc.tensor.matmul(
                            dst,
                            pt8_t[qt][:, 2 * kp:2 * kp + 2,
                                      sub * 128:(sub + 1) * 128],
                            ones16m[:, :, :],
                            start=(kp == 0), stop=(kp == NKP - 1),
                            perf_mode=DR)
                rq = small.tile([128, CCH], f32, tag="rq", name=f"rq{qt}")
                for sub in range(CCH):
                    nc.vector.reciprocal(
                        rq[:, sub:sub + 1],
                        dn[sub // 2][:, sub % 2, 0:1])
                rq_t[qt] = rq

            # tail: PV + epilogue for the final tile
            pv_t[3] = (pvA.tile([128, 2, KBLK], f32, tag="pvA", name="pva3"),
                       pvB.tile([128, 2, KBLK], f32, tag="pvB", name="pvb3"))
            pv_steps(pt8_t[3], *pv_t[3], range(NKP))
            epilogue(3, rq_t[3], *pv_t[3])

    nc.compile()
    return nc


def _get_nc():
    if "nc" not in _BUILD_CACHE:
        _BUILD_CACHE["nc"] = _build_nc()
    return _BUILD_CACHE["nc"]


def kernel(inputs, gamma, beta, wq, bq, wk, bk, wv, bv, wo, bo):
    from concourse.bass_utils import run_bass_kernel_spmd

    inputs = np.asarray(inputs, dtype=np.float32)
    gamma = np.asarray(gamma, dtype=np.float32)
    beta = np.asarray(beta, dtype=np.float32)
    wq = np.asarray(wq, dtype=np.float32)
    wk = np.asarray(wk, dtype=np.float32)
    wv = np.asarray(wv, dtype=np.float32)
    wo = np.asarray(wo, dtype=np.float32)
    bq = np.asarray(bq, dtype=np.float32)
    bk = np.asarray(bk, dtype=np.float32)
    bv = np.asarray(bv, dtype=np.float32)
    bo = np.asarray(bo, dtype=np.float32)

    # bq/bk shift the pre-softmax scores; per-query components cancel in the
    # softmax, and for this problem both are identically zero.
    assert np.abs(bq).max() == 0.0 and np.abs(bk).max() == 0.0, \
        "kernel assumes zero q/k biases"

    bf16 = ml_dtypes.bfloat16
    f8 = ml_dtypes.float8_e4m3
    # attn @ (V + 1*bv) = attn @ V + 1*bv  (attn rows sum to 1), so the
    # bias row (bv @ wo + bo) is added once in the residual term.
    brow = (bv.astype(np.float64) @ wo.astype(np.float64)).astype(np.float32) \
        + bo
    # fold the output projection into the value projection (associativity)
    # and the key projection into the query side: S = xn @ (wq@wk^T) @ xn^T.
    # Both folded weights are scaled x16 so their entries (~N(0,1/C)) sit in
    # the fp8e4 normal range; the exp scale and the 16.0-denominator operand
    # compensate exactly.
    wvo = (wv.astype(np.float64) @ wo.astype(np.float64)) * WSC
    wqk = (wq.astype(np.float64) @ wk.astype(np.float64).T) * WSC
    wqk8 = np.clip(wqk, -240, 240).astype(f8)
    wvo8 = np.clip(wvo, -240, 240).astype(f8)

    gmat = np.zeros((128, 8), np.float32)
    gmat[np.arange(128), np.arange(128) // GSIZE] = 1.0 / (GSIZE * N)
    gtmat = np.ascontiguousarray(
        (gmat.T > 0).astype(np.float32))
    imat = np.eye(128, dtype=np.float32).astype(bf16)

    x = inputs.reshape(B, N, C)
    in_maps = []
    for core in range(NCORES):
        b, h = divmod(core, 2)
        q0 = h * NQ
        rows = x[b]
        # queries first; key order is irrelevant (softmax is permutation
        # invariant over keys, and GroupNorm stats span the whole sample)
        perm = np.concatenate([rows[q0:q0 + NQ], rows[:q0], rows[q0 + NQ:]],
                              axis=0)
        perm8 = np.clip(perm, -240, 240).astype(f8)
        # xt: [tb, partition, cc, 512 tokens] (2KB contiguous per partition)
        xt_l = np.ascontiguousarray(
            perm8.T.reshape(CCH, 128, 8, 512).transpose(2, 1, 0, 3))
        in_maps.append({
            "xt": xt_l,
            "xrow": np.ascontiguousarray(
                perm8.reshape(NKC, 128, C).transpose(1, 0, 2)),
            "xr": (rows[q0:q0 + NQ] + brow[None, :]).astype(bf16),
            "wq": wqk8.reshape(CCH, 128, C).transpose(1, 0, 2).copy(),
            "wv": wvo8.reshape(CCH, 128, C).transpose(1, 0, 2).copy(),
            "gamma": gamma, "beta": beta,
            "gmat": gmat, "gtmat": gtmat, "imat": imat,
        })

    nc = _get_nc()
    res = run_bass_kernel_spmd(nc, in_maps, core_ids=list(range(NCORES)))

    out = np.empty((B, N, C), dtype=np.float32)
    for core in range(NCORES):
        b, h = divmod(core, 2)
        q0 = h * NQ
        out[b, q0:q0 + NQ] = res.results[core]["out"].astype(np.float32)
    return out.reshape(B, H, W, C)


if __name__ == "__main__":
    rng = np.random.default_rng(0)
    demo = {
        "inputs": rng.standard_normal((B, H, W, C), dtype=np.float32),
        "gamma": np.ones(C, np.float32), "beta": np.zeros(C, np.float32),
        "wq": rng.standard_normal((C, C)).astype(np.float32) / math.sqrt(C),
        "bq": np.zeros(C, np.float32),
        "wk": rng.standard_normal((C, C)).astype(np.float32) / math.sqrt(C),
        "bk": np.zeros(C, np.float32),
        "wv": rng.standard_normal((C, C)).astype(np.float32) / math.sqrt(C),
        "bv": np.zeros(C, np.float32),
        "wo": rng.standard_normal((C, C)).astype(np.float32) / math.sqrt(C),
        "bo": np.zeros(C, np.float32),
    }
    o = kernel(**demo)
    print("kernel output:", o.shape, o.dtype)


# revision 38
# speedup vs baseline: 1.0442x; 1.0442x over previous
"""TRN2 Bass/Tile kernel for AttentionBlock: GroupNorm(32) + 1x1-conv QKV +
single-head softmax attention over N=H*W tokens + output proj + residual.

Sharding: 8 cores = 4 samples x 2 query-halves (data parallel over batch,
query-parallel within sample). Each core receives the full (row-permuted)
sample so it can compute K/V for all 4096 tokens, but computes Q / attention /
output only for its 2048 query rows. No collectives needed.

v3: all four big GEMMs (Q-proj, V-proj, scores, PV) are fp8 (e4m3) DoubleRow
matmuls (two 128-deep k-planes per instruction, 0.5 cycles per output
column). The host supplies raw x in fp8 twice — channel-major x^T for GEMM
operands and token-major rows for statistics — plus x16-scaled fp8 folded
weights (wq@wk^T, wv@wo).

GroupNorm statistics run on the tensor engine while input DMAs stream:
sum(x) via an all-ones moving operand, sum(x^2) as the diagonal of per-chunk
Gram matrices X^T X (extracted with an identity mask + free-axis reduce).
rstd = exp(-0.5*ln(var+eps)) keeps the whole kernel on one activation table
(ln/exp/identity). The affine+fp8 quantize pass is token-major on DVE+Pool
so consumers start after the first 512 tokens.

Attention is a software pipeline over 512-query tiles: scores land in
two-bank PSUM pairs (one 1024-wide exp per pair keeps ACT saturated); P@V
for tile qt runs as a no-wait burst early in tile qt+1's window (qt3: second
half of its own window) into four persistent PSUM chains; the softmax
denominator is four tiny DoubleRow chains against a 16.0-constant moving
operand (the 16 folds the fp8 weight scale) read back per query partition
with a single reciprocal — no transposes. Normalization (1/(16 l)) and the
bf16 residual add fuse into one scalar_tensor_tensor per sub-tile.
"""

import math

import numpy as np
import ml_dtypes

B, H, W, C = 4, 64, 64, 512
N = H * W            # 4096 tokens per sample
NQ = N // 2          # 2048 query rows per core
GROUPS = 32
GSIZE = C // GROUPS  # 16 channels per group
EPS = 1e-5
NCORES = 8
KBLK = 512           # query-tile / psum free size
CCH = C // 128       # 4 channel chunks
NKC = N // 128       # 32 key chunks
NKP = NKC // 2       # 16 key chunk pairs
NQT = NQ // KBLK     # 4 query tiles
WSC = 16.0           # fp8 weight scale
EXP_SCALE = 1.0 / (WSC * math.sqrt(C))
EXP_BIAS = -2.0      # cancels in softmax; keeps exp() inside fp8e4 range
ONES_W = 64          # moving-operand width for the sum chains

_BUILD_CACHE = {}


def _build_nc():
    import concourse.bass as bass
    import concourse.tile as tile
    from concourse import bacc, mybir

    f32 = mybir.dt.float32
    bf16 = mybir.dt.bfloat16
    f8 = mybir.dt.float8e4
    Alu = mybir.AluOpType
    Act = mybir.ActivationFunctionType
    DR = mybir.MatmulPerfMode.DoubleRow

    nc = bacc.Bacc("TRN2", target_bir_lowering=False, debug=False,
                   num_devices=NCORES)

    xt_d = nc.dram_tensor("xt", [8, 128, CCH, 512], f8, kind="ExternalInput")
    xrow_d = nc.dram_tensor("xrow", [128, NKC, C], f8, kind="ExternalInput")
    xr_d = nc.dram_tensor("xr", [NQ, C], bf16, kind="ExternalInput")
    wq_d = nc.dram_tensor("wq", [128, CCH, C], f8, kind="ExternalInput")
    wv_d = nc.dram_tensor("wv", [128, CCH, C], f8, kind="ExternalInput")
    gamma_d = nc.dram_tensor("gamma", [C], f32, kind="ExternalInput")
    beta_d = nc.dram_tensor("beta", [C], f32, kind="ExternalInput")
    # gmat is pre-scaled by 1/(GSIZE*N) so the group matmul yields means
    gmat_d = nc.dram_tensor("gmat", [128, 8], f32, kind="ExternalInput")
    gtmat_d = nc.dram_tensor("gtmat", [8, 128], f32, kind="ExternalInput")
    imat_d = nc.dram_tensor("imat", [128, 128], bf16, kind="ExternalInput")
    out_d = nc.dram_tensor("out", [NQ, C], bf16, kind="ExternalOutput")

    with tile.TileContext(nc) as tc:
        with (
            tc.tile_pool(name="big", bufs=1) as big,
            tc.tile_pool(name="wpool", bufs=1) as wpool,
            tc.tile_pool(name="stats", bufs=1) as stats,
            tc.tile_pool(name="tmp", bufs=3) as tmp,
            tc.tile_pool(name="ptile", bufs=2) as ptile,
            tc.tile_pool(name="small", bufs=2) as small,
            tc.tile_pool(name="pp", bufs=2, space="PSUM") as pp,
            tc.tile_pool(name="pvA", bufs=1, space="PSUM") as pvA,
            tc.tile_pool(name="pvB", bufs=1, space="PSUM") as pvB,
        ):
            # ---- resident tensors (per-block tiles so consumers start as
            # soon as their block's DMA/affine lands) ----
            NTB = 8
            TB = N // NTB
            xt8_t = [big.tile([128, CCH, TB], f8, tag=f"xt8_{i}",
                              name=f"xt8_{i}") for i in range(NTB)]
            xrow8_h = [big.tile([128, NKC // 2, C], f8, tag=f"xrow8_{i}",
                                name=f"xrow8_{i}") for i in range(2)]
            xn8_t = [big.tile([128, CCH, TB], f8, tag=f"xn8_{i}",
                              name=f"xn8_{i}") for i in range(NTB)]

            def xn8s(ci0, ci1, n0, n1):
                t = n0 // TB
                assert n1 <= (t + 1) * TB
                return xn8_t[t][:, ci0:ci1, n0 - t * TB:n1 - t * TB]
            qt8 = big.tile([128, CCH, NQ], f8, tag="qt8")
            v8 = big.tile([128, NKC, C], f8, tag="v8")
            xr_sb = big.tile([128, NQ // 128, C], bf16, tag="xr")

            gamma_sb = wpool.tile([128, CCH], f32, tag="gamma")
            beta_sb = wpool.tile([128, CCH], f32, tag="beta")
            g_sb = wpool.tile([128, 8], f32, tag="gmat")
            gt_sb = wpool.tile([8, 128], f32, tag="gtmat")
            i_sb = wpool.tile([128, 128], bf16, tag="imat")

            # constants + ln/exp activation-table preload while DMAs run
            eps8 = wpool.tile([8, 1], f32, tag="eps")
            nc.vector.memset(eps8[:, :], EPS)
            bneg2 = wpool.tile([128, 1], f32, tag="bneg2")
            nc.vector.memset(bneg2[:, :], EXP_BIAS)
            ones1m = wpool.tile([128, 2, ONES_W], f8, tag="ones1m")
            nc.vector.memset(ones1m[:, :, :], 1.0)
            ones16m = wpool.tile([128, 2, ONES_W], f8, tag="ones16m")
            nc.vector.memset(ones16m[:, :, :], WSC)
            tjunk = wpool.tile([8, 1], f32, tag="tjunk")
            nc.scalar.activation(out=tjunk[:, :], in_=eps8[:, :], func=Act.Ln,
                                 bias=eps8[:, :], scale=1.0)

            # ---- input DMAs in shared-DMA-device priority order ----
            # xrow first (stats), then x^T token-blocks (affine), weights, xr
            qs_ = (nc.sync, nc.scalar)
            for hh in range(2):
                qs_[hh].dma_start(
                    out=xrow8_h[hh][:, :, :],
                    in_=xrow_d[:, hh * (NKC // 2):(hh + 1) * (NKC // 2), :])
            nc.sync.dma_start(out=gamma_sb[:, :],
                              in_=gamma_d.ap().rearrange("(a b) -> b a", b=128))
            nc.scalar.dma_start(out=beta_sb[:, :],
                               in_=beta_d.ap().rearrange("(a b) -> b a", b=128))
            nc.sync.dma_start(out=g_sb[:, :], in_=gmat_d[:, :])
            nc.scalar.dma_start(out=gt_sb[:, :], in_=gtmat_d[:, :])
            nc.sync.dma_start(out=i_sb[:, :], in_=imat_d[:, :])
            for tb in range(NTB):
                qs_[tb % 2].dma_start(out=xt8_t[tb][:, :, :],
                                      in_=xt_d[tb, :, :, :])
            w8 = {}
            for qi, (name, wd) in enumerate((("wq", wq_d), ("wv", wv_d))):
                w8[name] = wpool.tile([128, CCH, C], f8, tag=name,
                                      name=f"w_{name}")
                qs_[qi].dma_start(out=w8[name][:, :, :], in_=wd[:, :, :])
            for i in range(4):
                qs_[i % 2].dma_start(
                    out=xr_sb[:, i * 4:(i + 1) * 4, :],
                    in_=xr_d.ap().rearrange("(a b) d -> b a d", b=128)[
                        :, i * 4:(i + 1) * 4, :])

            # ---- GroupNorm statistics on the PE while DMAs stream ----
            # sum(x): ones-moving DoubleRow chains; sum(x^2): Gram diagonals.
            # Each chunk chain owns one psum bank (a pool-tile half).
            mv2 = stats.tile([128, CCH, 2], f32, tag="mv2")  # (sum, sumsq)
            sxt = [pp.tile([128, 2, KBLK], f32, tag="pp", name=f"sx{i}")
                   for i in range(2)]
            gmt = [pvA.tile([128, 2, KBLK], f32, tag="pvA", name="gram01"),
                   pvB.tile([128, 2, KBLK], f32, tag="pvB", name="gram23")]
            for cc in range(CCH):
                psx = sxt[cc // 2][:, cc % 2, 0:ONES_W]
                psg = gmt[cc // 2][:, cc % 2, 0:128]
                for nbp in range(NKP):
                    hh, off = divmod(2 * nbp, NKC // 2)
                    lhs = xrow8_h[hh][:, off:off + 2,
                                      cc * 128:(cc + 1) * 128]
                    nc.tensor.matmul(psx, lhs, ones1m[:, :, :],
                                     start=(nbp == 0), stop=(nbp == NKP - 1),
                                     perf_mode=DR)
                    nc.tensor.matmul(psg, lhs, lhs,
                                     start=(nbp == 0), stop=(nbp == NKP - 1),
                                     perf_mode=DR)
            djunk = stats.tile([128, 128], f32, tag="djunk")
            for cc in range(CCH):
                nc.vector.tensor_copy(mv2[:, cc, 0:1],
                                      sxt[cc // 2][:, cc % 2, 0:1])
                nc.vector.tensor_tensor(out=djunk[:, :],
                                        in0=gmt[cc // 2][:, cc % 2, 0:128],
                                        in1=i_sb[:, :], op=Alu.mult)
                nc.vector.tensor_reduce(out=mv2[:, cc, 1:2], in_=djunk[:, :],
                                        axis=mybir.AxisListType.X, op=Alu.add)

            # group combine: per-cc [8,2] matmuls, one bank per chain; the
            # host-scaled gmat turns sums directly into (mean, E[x^2])
            cmb = [pp.tile([128, 2, KBLK], f32, tag="pp", name=f"cmb{i}")
                   for i in range(2)]
            sg = stats.tile([8, CCH, 2], f32, tag="sg")
            for cc in range(CCH):
                pcc = cmb[cc // 2][0:8, cc % 2, 0:2]
                nc.tensor.matmul(pcc, g_sb[:, :], mv2[:, cc, :],
                                 start=True, stop=True)
                nc.vector.tensor_copy(sg[:, cc, :], pcc)
            gv = stats.tile([8, CCH], f32, tag="gv")
            nc.vector.tensor_mul(gv[:, :], sg[:, :, 0], sg[:, :, 0])
            nc.vector.tensor_sub(gv[:, :], sg[:, :, 1], gv[:, :])
            # rstd = exp(-0.5*ln(var+eps)) — stays on the ln/exp ACT table
            nc.scalar.activation(out=gv[:, :], in_=gv[:, :], func=Act.Ln,
                                 bias=eps8[:, :], scale=1.0)
            nc.scalar.activation(out=sg[:, :, 1], in_=gv[:, :], func=Act.Exp,
                                 scale=-0.5)
            # broadcast mean/rstd back to channel partitions (two chains)
            mbp = pvA.tile([128, 2, KBLK], f32, tag="pvA", name="mb")
            nc.tensor.matmul(mbp[:, 0, 0:CCH], gt_sb[:, :], sg[:, :, 0],
                             start=True, stop=True)
            nc.tensor.matmul(mbp[:, 1, 0:CCH], gt_sb[:, :], sg[:, :, 1],
                             start=True, stop=True)
            a_sb = stats.tile([128, CCH], f32, tag="A")
            b_sb = stats.tile([128, CCH], f32, tag="Bb")
            nc.vector.tensor_mul(a_sb[:, :], mbp[:, 1, 0:CCH], gamma_sb[:, :])
            nc.vector.tensor_mul(b_sb[:, :], mbp[:, 0, 0:CCH], a_sb[:, :])
            nc.vector.tensor_sub(b_sb[:, :], beta_sb[:, :], b_sb[:, :])

            # keep the PE p-state warm between the stats chains and the
            # first score matmuls (dummy DoubleRow chain, no consumers)
            warm = pvA.tile([128, 2, KBLK], f32, tag="pvA", name="warm")
            for i in range(8):
                nc.tensor.matmul(warm[:, 0, :],
                                 xrow8_h[0][:, 0:2, 0:128],
                                 xrow8_h[0][:, 0:2, :],
                                 start=(i == 0), stop=(i == 7),
                                 perf_mode=DR)

            # ---- affine + fp8 quantize, token-major, Pool-heavy so DVE
            # stays free for the projection-psum copies ----
            for tb in range(NTB):
                for cc in range(CCH):
                    eng = nc.vector if cc == 0 else nc.gpsimd
                    eng.tensor_scalar(
                        out=xn8_t[tb][:, cc, :], in0=xt8_t[tb][:, cc, :],
                        scalar1=a_sb[:, cc:cc + 1],
                        scalar2=b_sb[:, cc:cc + 1],
                        op0=Alu.mult, op1=Alu.add)

            # ---- helper emitters (DoubleRow fp8 everywhere) ----
            # Projection psums go through the pvA/pvB rings (idle until the
            # first P@V in window 1) so the scores/exp stream owns the pp
            # ring exclusively; each pair-tile is drained by ONE wide copy.
            def qproj(qtile, half, pool, tag, on_act=False):
                """Project 512 queries for dc chunks (2*half, 2*half+1)."""
                q0 = qtile * KBLK
                pt_ = pool.tile([128, 2, KBLK], f32, tag=tag,
                                name=f"qp{qtile}_{half}")
                for h2 in range(2):
                    dc = 2 * half + h2
                    for ci in range(0, CCH, 2):
                        nc.tensor.matmul(
                            pt_[:, h2, :],
                            w8["wq"][:, ci:ci + 2,
                                     dc * 128:(dc + 1) * 128],
                            xn8s(ci, ci + 2, q0, q0 + KBLK),
                            start=(ci == 0), stop=(ci == CCH - 2),
                            perf_mode=DR)
                dst = qt8[:, 2 * half:2 * half + 2, q0:q0 + KBLK]
                if on_act:
                    nc.scalar.activation(out=dst, in_=pt_[:, :, :],
                                         func=Act.Identity)
                else:
                    nc.vector.tensor_copy(dst, pt_[:, :, :])

            def vproj_pair(i, pool, tag):
                """V-projection for token blocks 2i, 2i+1 (halves of a tile)."""
                pt_ = pool.tile([128, 2, KBLK], f32, tag=tag,
                                name=f"vp{i}")
                for h2 in range(2):
                    nb = 2 * i + h2
                    for ci in range(0, CCH, 2):
                        nc.tensor.matmul(
                            pt_[:, h2, :],
                            xn8s(ci, ci + 2, nb * 128, (nb + 1) * 128),
                            w8["wv"][:, ci:ci + 2, :],
                            start=(ci == 0), stop=(ci == CCH - 2),
                            perf_mode=DR)
                nc.vector.tensor_copy(v8[:, 2 * i:2 * i + 2, :],
                                      pt_[:, :, :])

            def pv_steps(pt8_t, psA, psB, kps):
                """P@V chain steps (4 sub chains, kp-major)."""
                for kp in kps:
                    for sub in range(CCH):
                        dst = (psA, psB)[sub // 2][:, sub % 2, :]
                        nc.tensor.matmul(
                            dst,
                            pt8_t[:, 2 * kp:2 * kp + 2,
                                  sub * 128:(sub + 1) * 128],
                            v8[:, 2 * kp:2 * kp + 2, :],
                            start=(kp == 0), stop=(kp == NKP - 1),
                            perf_mode=DR)

            def epilogue(qt, rq, psA, psB):
                q0 = qt * KBLK
                for sub in range(CCH):
                    src = (psA, psB)[sub // 2][:, sub % 2, :]
                    res = tmp.tile([128, C], bf16, tag="res",
                                   name=f"res{qt}_{sub}")
                    nc.vector.scalar_tensor_tensor(
                        out=res[:, :], in0=src,
                        scalar=rq[:, sub:sub + 1],
                        in1=xr_sb[:, qt * 4 + sub, :],
                        op0=Alu.mult, op1=Alu.add)
                    qs = slice(q0 + sub * 128, q0 + (sub + 1) * 128)
                    nc.sync.dma_start(out=out_d[qs, :], in_=res[:, :])

            # ---- attention: software pipeline over query tiles ----
            qproj(0, 0, pp, "pp", on_act=False)
            qproj(0, 1, pp, "pp", on_act=True)
            # filler queue: projection tiles drained through the pvA/pvB
            # rings during windows 0-1 (one per slot, alternating rings)
            fillers = []
            for qtile in (1, 2, 3):
                for half in range(2):
                    fillers.append(("q", qtile, half))
            for i in range(NKC // 2):
                fillers.append(("v", i))
            fi = 0
            pt8_t = {}
            rq_t = {}
            pv_t = {}
            for qt in range(NQT):
                q0 = qt * KBLK
                pt8_t[qt] = ptile.tile([128, NKC, KBLK], f8, tag="pt",
                                       name=f"pt{qt}")
                for kp in range(NKP):
                    # PV steps for the previous tile first (always ready);
                    # qt0's V copies drain slowly, so window 1 paces 1/slot,
                    # later windows front-load 2/slot and free the PV psums
                    # by mid-window
                    if qt >= 1:
                        prev = qt - 1
                        if kp == 0:
                            pv_t[prev] = (
                                pvA.tile([128, 2, KBLK], f32, tag="pvA",
                                         name=f"pva{prev}"),
                                pvB.tile([128, 2, KBLK], f32, tag="pvB",
                                         name=f"pvb{prev}"))
                        if qt == 1:
                            pv_steps(pt8_t[prev], *pv_t[prev], [kp])
                        elif kp < 8:
                            pv_steps(pt8_t[prev], *pv_t[prev], [2 * kp,
                                                                2 * kp + 1])

                    ppt = pp.tile([128, 2, KBLK], f32, tag="pp",
                                  name=f"s{qt}_{kp}")
                    for half in range(2):
                        kc = 2 * kp + half
                        for ci in range(0, CCH, 2):
                            nc.tensor.matmul(
                                ppt[:, half, :],
                                xn8s(ci, ci + 2, kc * 128, (kc + 1) * 128),
                                qt8[:, ci:ci + 2, q0:q0 + KBLK],
                                start=(ci == 0), stop=(ci == CCH - 2),
                                perf_mode=DR)
                    nc.scalar.activation(out=pt8_t[qt][:, 2 * kp:2 * kp + 2, :],
                                         in_=ppt[:, :, :], func=Act.Exp,
                                         scale=EXP_SCALE, bias=bneg2[:, :])

                    # one filler tile per slot while windows 0-1 have psum
                    # slack in the PV rings
                    if qt == 0 and fi < len(fillers):
                        for _ in range(2):
                            if fi < len(fillers):
                                f = fillers[fi]
                                pool, tag = ((pvA, "pvA") if fi % 2 == 0
                                             else (pvB, "pvB"))
                                if f[0] == "q":
                                    qproj(f[1], f[2], pool, tag)
                                else:
                                    vproj_pair(f[1], pool, tag)
                                fi += 1
                    if qt == 1 and kp == NKP - 1:
                        epilogue(qt - 1, rq_t[qt - 1], *pv_t[qt - 1])
                    elif qt >= 2 and kp == 9:
                        epilogue(qt - 1, rq_t[qt - 1], *pv_t[qt - 1])

                # denominator: four tiny chains against the 16.0 operand,
                # emitted at tile end (all exps of qt done shortly after)
                dn = [pp.tile([128, 2, KBLK], f32, tag="pp",
                              name=f"dn{qt}_{i}") for i in range(2)]
                for sub in range(CCH):
                    dst = dn[sub // 2][:, sub % 2, 0:ONES_W]
                    for kp in range(NKP):
                        nc.tensor.matmul(
                            dst,
                            pt8_t[qt][:, 2 * kp:2 * kp + 2,
                                      sub * 128:(sub + 1) * 128],
                            ones16m[:, :, :],
                            start=(kp == 0), stop=(kp == NKP - 1),
                            perf_mode=DR)
                rq = small.tile([128, CCH], f32, tag="rq", name=f"rq{qt}")
                for sub in range(CCH):
                    nc.vector.reciprocal(
                        rq[:, sub:sub + 1],
                        dn[sub // 2][:, sub % 2, 0:1])
                rq_t[qt] = rq

            # tail: PV + epilogue for the final tile
            pv_t[3] = (pvA.tile([128, 2, KBLK], f32, tag="pvA", name="pva3"),
                       pvB.tile([128, 2, KBLK], f32, tag="pvB", name="pvb3"))
            pv_steps(pt8_t[3], *pv_t[3], range(NKP))
            epilogue(3, rq_t[3], *pv_t[3])

    nc.compile()
    return nc


def _get_nc():
    if "nc" not in _BUILD_CACHE:
        _BUILD_CACHE["nc"] = _build_nc()
    return _BUILD_CACHE["nc"]


def kernel(inputs, gamma, beta, wq, bq, wk, bk, wv, bv, wo, bo):
    from concourse.bass_utils import run_bass_kernel_spmd

    inputs = np.asarray(inputs, dtype=np.float32)
    gamma = np.asarray(gamma, dtype=np.float32)
    beta = np.asarray(beta, dtype=np.float32)
    wq = np.asarray(wq, dtype=np.float32)
    wk = np.asarray(wk, dtype=np.float32)
    wv = np.asarray(wv, dtype=np.float32)
    wo = np.asarray(wo, dtype=np.float32)
    bq = np.asarray(bq, dtype=np.float32)
    bk = np.asarray(bk, dtype=np.float32)
    bv = np.asarray(bv, dtype=np.float32)
    bo = np.asarray(bo, dtype=np.float32)

    # bq/bk shift the pre-softmax scores; per-query components cancel in the
    # softmax, and for this problem both are identically zero.
    assert np.abs(bq).max() == 0.0 and np.abs(bk).max() == 0.0, \
        "kernel assumes zero q/k biases"

    bf16 = ml_dtypes.bfloat16
    f8 = ml_dtypes.float8_e4m3
    # attn @ (V + 1*bv) = attn @ V + 1*bv  (attn rows sum to 1), so the
    # bias row (bv @ wo + bo) is added once in the residual term.
    brow = (bv.astype(np.float64) @ wo.astype(np.float64)).astype(np.float32) \
        + bo
    # fold the output projection into the value projection (associativity)
    # and the key projection into the query side: S = xn @ (wq@wk^T) @ xn^T.
    # Both folded weights are scaled x16 so their entries (~N(0,1/C)) sit in
    # the fp8e4 normal range; the exp scale and the 16.0-denominator operand
    # compensate exactly.
    wvo = (wv.astype(np.float64) @ wo.astype(np.float64)) * WSC
    wqk = (wq.astype(np.float64) @ wk.astype(np.float64).T) * WSC
    wqk8 = np.clip(wqk, -240, 240).astype(f8)
    wvo8 = np.clip(wvo, -240, 240).astype(f8)

    gmat = np.zeros((128, 8), np.float32)
    gmat[np.arange(128), np.arange(128) // GSIZE] = 1.0 / (GSIZE * N)
    gtmat = np.ascontiguousarray(
        (gmat.T > 0).astype(np.float32))
    imat = np.eye(128, dtype=np.float32).astype(bf16)

    x = inputs.reshape(B, N, C)
    in_maps = []
    for core in range(NCORES):
        b, h = divmod(core, 2)
        q0 = h * NQ
        rows = x[b]
        # queries first; key order is irrelevant (softmax is permutation
        # invariant over keys, and GroupNorm stats span the whole sample)
        perm = np.concatenate([rows[q0:q0 + NQ], rows[:q0], rows[q0 + NQ:]],
                              axis=0)
        perm8 = np.clip(perm, -240, 240).astype(f8)
        # xt: [tb, partition, cc, 512 tokens] (2KB contiguous per partition)
        xt_l = np.ascontiguousarray(
            perm8.T.reshape(CCH, 128, 8, 512).transpose(2, 1, 0, 3))
        in_maps.append({
            "xt": xt_l,
            "xrow": np.ascontiguousarray(
                perm8.reshape(NKC, 128, C).transpose(1, 0, 2)),
            "xr": (rows[q0:q0 + NQ] + brow[None, :]).astype(bf16),
            "wq": wqk8.reshape(CCH, 128, C).transpose(1, 0, 2).copy(),
            "wv": wvo8.reshape(CCH, 128, C).transpose(1, 0, 2).copy(),
            "gamma": gamma, "beta": beta,
            "gmat": gmat, "gtmat": gtmat, "imat": imat,
        })

    nc = _get_nc()
    res = run_bass_kernel_spmd(nc, in_maps, core_ids=list(range(NCORES)))

    out = np.empty((B, N, C), dtype=np.float32)
    for core in range(NCORES):
        b, h = divmod(core, 2)
        q0 = h * NQ
        out[b, q0:q0 + NQ] = res.results[core]["out"].astype(np.float32)
    return out.reshape(B, H, W, C)


if __name__ == "__main__":
    rng = np.random.default_rng(0)
    demo = {
        "inputs": rng.standard_normal((B, H, W, C), dtype=np.float32),
        "gamma": np.ones(C, np.float32), "beta": np.zeros(C, np.float32),
        "wq": rng.standard_normal((C, C)).astype(np.float32) / math.sqrt(C),
        "bq": np.zeros(C, np.float32),
        "wk": rng.standard_normal((C, C)).astype(np.float32) / math.sqrt(C),
        "bk": np.zeros(C, np.float32),
        "wv": rng.standard_normal((C, C)).astype(np.float32) / math.sqrt(C),
        "bv": np.zeros(C, np.float32),
        "wo": rng.standard_normal((C, C)).astype(np.float32) / math.sqrt(C),
        "bo": np.zeros(C, np.float32),
    }
    o = kernel(**demo)
    print("kernel output:", o.shape, o.dtype)
